# revision 3
# baseline (speedup 1.0000x reference)
"""Trainium2 Bass kernel for nn_DecouplingFlowLayer.

Computes, for x [B=4, S=128, N=512, F=362] fp32:
  X_l_proj = (x with feature0 := Haar-lowpass)  @ Wg^T + Wg_b   -> [B,S,N,64]
  X_h_proj = (x with feature0 := Haar-highpass) @ Wh^T + Wh_b   -> [B,S,N,64]

Strategy (per NeuronCore, data-parallel over B*S across 8 cores), v2:
  - One "tile" = one (b,s) slice = 512 tokens x 362 features (contiguous in
    HBM).  Token t of a slice sits at SBUF partition t//4, chunk t%4 so that
    output stores stay >=512B contiguous per partition.
  - The whole datapath is fp16 (tolerance is 2e-2; fp16 keeps us ~1e-3):
      * Pool engine up-converts x fp32 -> fp16 (it is otherwise idle and has
        no other role; DVE/ACT carry the PSUM copies Pool cannot do).
      * fp16 PE transposes run 1 cyc/row (fp32: 2) and their LDWEIGHTS get
        the compiler's fast-weight-load path (fp16 + 128 cols).
  - K is padded 362 -> 384: col 362/363 = Haar pair terms (rank-1 update
    that produces the feature-0 replacement for both outputs at once),
    col 364 = ones (fuses the bias into the GEMM), cols 365.. = zeros.
  - GEMM is "flipped": stationary = transposed x chunk [f,128tok], moving =
    W block [f,128] -> PSUM [tok, d] directly; no output transpose and a
    single PSUM->SBUF fp16 copy per slice.
  - Outputs are stored fp16 (halves write traffic; DMA is the roofline);
    the host widens to fp32.
"""

import numpy as np

import concourse.bass as bass
import concourse.mybir as mybir
from concourse.bass_utils import run_bass_kernel_spmd
from concourse.tile import TileContext

F32 = mybir.dt.float32
F16 = mybir.dt.float16

N_CORES = 8
B, S, N, F, D = 4, 128, 512, 362, 64
BS = B * S                     # 512 (b,s) slices
TPC = BS // N_CORES            # 64 slices per core
FW2 = 384                      # padded K: 362 data + 2 haar + 1 ones + 19 zero
NK = 3                         # K blocks of 128


def _patch_drain():
    """walrus (TRN2) can encode only one sync-wait per instruction for several
    instruction formats (Matmult/S3_LW, SP CTRL drain, ...). Tile's scheduler
    happily attaches 2+ waits. Hoist excess waits onto standalone
    InstEventSemaphore instructions on the same engine (identical sequencer
    stall semantics), keeping one wait on the original instruction."""
    import concourse.tile as tile_mod
    from concourse.vector_clock import ScopedClock

    if getattr(tile_mod.TileContext, "_drain_split_patch", False):
        return

    orig_cal = tile_mod.TileContext._commit_and_lower

    def _commit_and_lower(self, inst, original_block, old_bb_map, bb_to_exit_bb):
        si = getattr(inst, "sync_info", None)
        waits = list(si.on_wait) if (si and si.on_wait) else []
        if (
            len(waits) > 1
            and isinstance(inst, mybir.Instruction)
            and inst.engine != mybir.EngineType.Unassigned
            and not type(inst).__name__.startswith("BassTile")
        ):
            for w in waits[:-1]:
                ev = mybir.InstEventSemaphore(
                    name=f"EVW-{self.nc.next_id()}",
                    ins=[],
                    outs=[],
                    sync_info=mybir.SyncInfo(on_wait=[w], on_update=[]),
                )
                ev.engine = inst.engine
                orig_cal(self, ev, original_block, old_bb_map, bb_to_exit_bb)
            inst.sync_info = mybir.SyncInfo(
                on_wait=[waits[-1]], on_update=list(si.on_update or [])
            )
        return orig_cal(self, inst, original_block, old_bb_map, bb_to_exit_bb)

    tile_mod.TileContext._commit_and_lower = _commit_and_lower

    def _drain_and_barrier(self, tick_clock, wait_clock):
        nc = self.nc
        drain_inst = nc.sync.drain()
        wait_clock.add_sem_waits(
            drain_inst.ins, ScopedClock({None: tick_clock.global_clock})
        )
        si = drain_inst.ins.sync_info
        waits = list(si.on_wait or [])
        if len(waits) > 1:
            drain_inst.ins.sync_info = mybir.SyncInfo(
                on_wait=waits[:1], on_update=list(si.on_update or [])
            )
            for i in range(1, len(waits)):
                extra = nc.sync.drain()
                extra.ins.sync_info = mybir.SyncInfo(
                    on_wait=waits[i : i + 1], on_update=[]
                )
        nc.all_engine_barrier()
        assert self.sems is not None
        popped = nc._tile_sem_poison_stack.pop()
        assert popped is self._sem_poison
        nc.clear_and_free_semaphores(list(self.sems.allocated().values()))
        nc.all_engine_barrier()

    tile_mod.TileContext._drain_and_barrier = _drain_and_barrier
    tile_mod.TileContext._drain_split_patch = True


def _patch_birsim_off():
    """The walrus BIR-simulation pass re-executes every instruction on host
    and dominates compile time (~19 min for this kernel vs <1 s without).
    It is a validation-only pass; disable it for our compiles."""
    import concourse.bass_utils as bu

    if getattr(bu, "_birsim_off_patch", False):
        return
    orig = bu.bir_verify_and_optimise

    def patched(tmpdir, inp="bir.json", outp="file.neff", arch=None, *, dve_root=None):
        real_run = bu.run_command

        def run_hook(cmd, **kw):
            cmd = [
                "--enable-birsim=false" if c == "--enable-birsim=true" else c
                for c in cmd
            ]
            return real_run(cmd, **kw)

        bu.run_command = run_hook
        try:
            return orig(tmpdir, inp, outp, arch, dve_root=dve_root)
        finally:
            bu.run_command = real_run

    bu.bir_verify_and_optimise = patched
    bu._birsim_off_patch = True


def _build_nc():
    _patch_drain()
    _patch_birsim_off()
    nc = bass.Bass("TRN2", target_bir_lowering=False, debug=False)

    # token t of a slice = (partition t//4, chunk t%4)
    x_d = nc.declare_dram_parameter("x", [TPC, 128, 4, F], F32, isOutput=False)
    w_d = nc.declare_dram_parameter("w", [NK, 128, 128], F16, isOutput=False)
    id_d = nc.declare_dram_parameter("ident", [128, 128], F16, isOutput=False)
    o_d = nc.declare_dram_parameter("out", [2, TPC, 512, 64], F16, isOutput=True)

    with TileContext(nc) as tc:
        with (
            tc.tile_pool(name="const", bufs=1) as cpool,
            tc.tile_pool(name="nat", bufs=3) as natp,
            tc.tile_pool(name="nath", bufs=2) as nhp,
            tc.tile_pool(name="xt", bufs=6) as xtp,
            tc.tile_pool(name="stg", bufs=2) as stgp,
            tc.tile_pool(name="pxt", bufs=6, space="PSUM") as pxtp,
            tc.tile_pool(name="pmm", bufs=2, space="PSUM") as pmmp,
        ):
            wsb = cpool.tile([128, NK, 128], F16, tag="w", name="w")
            nc.sync.dma_start(
                out=wsb[:, :, :], in_=w_d.rearrange("k p d -> p k d")
            )
            ident = cpool.tile([128, 128], F16, tag="ident", name="ident")
            nc.sync.dma_start(out=ident[:, :], in_=id_d[:, :])

            def load_quad(qd):
                nat = natp.tile([128, 4, 4, F], F32, tag="nat", name="nat")
                nc.sync.dma_start(
                    out=nat[:, :, :, :],
                    in_=x_d[4 * qd : 4 * qd + 4].rearrange("t p c f -> p t c f"),
                )
                nath = nhp.tile([128, 4, 4, FW2], F16, tag="nath", name="nath")
                # ones column fuses the bias into the GEMM; zero tail pads K
                nc.gpsimd.memset(nath[:, :, :, F + 2 : F + 3], 1.0)
                nc.gpsimd.memset(nath[:, :, :, F + 3 : FW2], 0.0)
                for pp in range(2):
                    e, o = 2 * pp, 2 * pp + 1
                    x0e, x0o = nat[:, e, :, 0], nat[:, o, :, 0]
                    # col 362: (pair - self); col 363: (self + pair)
                    nc.vector.tensor_sub(nath[:, e, :, F], x0o, x0e)
                    nc.vector.tensor_sub(nath[:, o, :, F], x0e, x0o)
                    nc.vector.tensor_add(nath[:, e, :, F + 1], x0e, x0o)
                    nc.vector.tensor_copy(nath[:, o, :, F + 1], nath[:, e, :, F + 1])
                nc.gpsimd.tensor_copy(nath[:, :, :, 0:F], nat[:, :, :, 0:F])
                return nath

            def trans_phase(nath, ti):
                xts = []
                for k in range(NK):
                    pxt = pxtp.tile([128, 512], F16, tag="pxt", name="pxt")
                    for c in range(4):
                        nc.tensor.transpose(
                            pxt[:, c * 128 : (c + 1) * 128],
                            nath[:, ti, c, k * 128 : (k + 1) * 128],
                            ident[:, :],
                        )
                    xtb = xtp.tile([128, 512], F16, tag="xt", name="xt")
                    if k == 1:
                        nc.scalar.copy(xtb[:, :], pxt[:, :])
                    else:
                        nc.vector.tensor_copy(xtb[:, :], pxt[:, :])
                    xts.append(xtb)
                return xts

            def gemm_phase(xts, stg, ti):
                pmm = pmmp.tile([128, 4, 128], F32, tag="pmm", name="pmm")
                for c in range(4):
                    for k in range(NK):
                        nc.tensor.matmul(
                            pmm[:, c, :],
                            xts[k][:, c * 128 : (c + 1) * 128],
                            wsb[:, k, :],
                            start=(k == 0),
                            stop=(k == NK - 1),
                        )
                nc.scalar.copy(
                    stg[:, ti],
                    pmm.rearrange("p q (lh d) -> p lh q d", lh=2),
                )

            def store_quad(qd, stg):
                for lh in range(2):
                    nc.scalar.dma_start(
                        out=o_d[lh, 4 * qd : 4 * qd + 4].rearrange(
                            "t (p q) d -> p t q d", q=4
                        ),
                        in_=stg[:, :, lh],
                    )

            # software pipeline: PE transposes slice t while slice t-1's GEMM
            # waits on its PSUM->SBUF casts
            stg_tiles = {}
            prev = None
            for t in range(TPC):
                qd, ti = divmod(t, 4)
                if ti == 0:
                    nath = load_quad(qd)
                    stg_tiles[qd] = stgp.tile(
                        [128, 4, 2, 4, 64], F16, tag="stg", name="stg"
                    )
                xts = trans_phase(nath, ti)
                if prev is not None:
                    pq, pt = divmod(t - 1, 4)
                    gemm_phase(prev, stg_tiles[pq], pt)
                    if pt == 3:
                        store_quad(pq, stg_tiles.pop(pq))
                prev = xts
            gemm_phase(prev, stg_tiles[TPC // 4 - 1], 3)
            store_quad(TPC // 4 - 1, stg_tiles.pop(TPC // 4 - 1))
    return nc


_NC = None


def kernel(x, Wg_w, Wg_b, Wh_w, Wh_b):
    global _NC
    if _NC is None:
        _NC = _build_nc()

    x = np.ascontiguousarray(np.asarray(x, dtype=np.float32))
    Wg_w = np.asarray(Wg_w, dtype=np.float32)
    Wg_b = np.asarray(Wg_b, dtype=np.float32)
    Wh_w = np.asarray(Wh_w, dtype=np.float32)
    Wh_b = np.asarray(Wh_b, dtype=np.float32)

    waug = np.zeros((FW2, 128), dtype=np.float32)
    waug[:F, :64] = Wg_w.T
    waug[:F, 64:] = Wh_w.T
    waug[F, :64] = 0.5 * Wg_w[:, 0]
    waug[F + 1, 64:] = -0.5 * Wh_w[:, 0]
    waug[F + 2, :64] = Wg_b
    waug[F + 2, 64:] = Wh_b
    waug = waug.reshape(NK, 128, 128).astype(np.float16)
    ident = np.eye(128, dtype=np.float16)

    xf = x.reshape(BS, N, F)
    in_maps = []
    for i in range(N_CORES):
        shard = xf[i * TPC : (i + 1) * TPC].reshape(TPC, 128, 4, F)
        in_maps.append({"x": shard, "w": waug, "ident": ident})

    res = run_bass_kernel_spmd(_NC, in_maps, list(range(N_CORES)))
    out_l = np.concatenate(
        [res.results[i]["out"][0] for i in range(N_CORES)], axis=0
    ).astype(np.float32).reshape(B, S, N, D)
    out_h = np.concatenate(
        [res.results[i]["out"][1] for i in range(N_CORES)], axis=0
    ).astype(np.float32).reshape(B, S, N, D)
    return (out_l, out_h)


# revision 7
# speedup vs baseline: 2.0452x; 2.0452x over previous
"""Trainium2 Bass kernel for nn_DecouplingFlowLayer.

Computes, for x [B=4, S=128, N=512, F=362] fp32:
  X_l_proj = (x with feature0 := Haar-lowpass)  @ Wg^T + Wg_b   -> [B,S,N,64]
  X_h_proj = (x with feature0 := Haar-highpass) @ Wh^T + Wh_b   -> [B,S,N,64]

Strategy (per NeuronCore, data-parallel over B*S across 8 cores), v2:
  - One "tile" = one (b,s) slice = 512 tokens x 362 features (contiguous in
    HBM).  Token t of a slice sits at SBUF partition t//4, chunk t%4 so that
    output stores stay >=512B contiguous per partition.
  - The whole datapath is fp16 (tolerance is 2e-2; fp16 keeps us ~1e-3):
      * the x load is a gpsimd (SWDGE) casting DMA: fp32 HBM -> fp16 SBUF
        during the transfer, so no on-chip convert pass exists at all.
      * fp16 PE transposes run 1 cyc/row (fp32: 2) and their LDWEIGHTS get
        the compiler's fast-weight-load path (fp16 + 128 cols).
  - K is padded 362 -> 384: col 362 = the paired slice's RAW feature 0 (the
    Haar avg/diff algebra is folded into W rows 0/362, uniformly for even
    and odd slices), col 363 = ones (fuses the bias into the GEMM),
    cols 364.. = zeros.
  - GEMM is "flipped": stationary = transposed x chunk [f,128tok], moving =
    W block [f,128] -> PSUM [tok, d] directly; no output transpose and a
    single PSUM->SBUF fp16 copy per slice.
  - Outputs are stored fp16 (halves write traffic; DMA is the roofline);
    the host widens to fp32.
"""

import numpy as np

import concourse.bass as bass
import concourse.mybir as mybir
from concourse.bass_utils import run_bass_kernel_spmd
from concourse.tile import TileContext

F32 = mybir.dt.float32
F16 = mybir.dt.float16

N_CORES = 8
B, S, N, F, D = 4, 128, 512, 362, 64
BS = B * S                     # 512 (b,s) slices
TPC = BS // N_CORES            # 64 slices per core
FW2 = 384                      # padded K: 362 data + 2 haar + 1 ones + 19 zero
NK = 3                         # K blocks of 128


def _patch_drain():
    """walrus (TRN2) can encode only one sync-wait per instruction for several
    instruction formats (Matmult/S3_LW, SP CTRL drain, ...). Tile's scheduler
    happily attaches 2+ waits. Hoist excess waits onto standalone
    InstEventSemaphore instructions on the same engine (identical sequencer
    stall semantics), keeping one wait on the original instruction."""
    import concourse.tile as tile_mod
    from concourse.vector_clock import ScopedClock

    if getattr(tile_mod.TileContext, "_drain_split_patch", False):
        return

    orig_cal = tile_mod.TileContext._commit_and_lower

    def _commit_and_lower(self, inst, original_block, old_bb_map, bb_to_exit_bb):
        si = getattr(inst, "sync_info", None)
        waits = list(si.on_wait) if (si and si.on_wait) else []
        if (
            len(waits) > 1
            and isinstance(inst, mybir.Instruction)
            and inst.engine != mybir.EngineType.Unassigned
            and not type(inst).__name__.startswith("BassTile")
        ):
            for w in waits[:-1]:
                ev = mybir.InstEventSemaphore(
                    name=f"EVW-{self.nc.next_id()}",
                    ins=[],
                    outs=[],
                    sync_info=mybir.SyncInfo(on_wait=[w], on_update=[]),
                )
                ev.engine = inst.engine
                orig_cal(self, ev, original_block, old_bb_map, bb_to_exit_bb)
            inst.sync_info = mybir.SyncInfo(
                on_wait=[waits[-1]], on_update=list(si.on_update or [])
            )
        return orig_cal(self, inst, original_block, old_bb_map, bb_to_exit_bb)

    tile_mod.TileContext._commit_and_lower = _commit_and_lower

    def _drain_and_barrier(self, tick_clock, wait_clock):
        nc = self.nc
        drain_inst = nc.sync.drain()
        wait_clock.add_sem_waits(
            drain_inst.ins, ScopedClock({None: tick_clock.global_clock})
        )
        si = drain_inst.ins.sync_info
        waits = list(si.on_wait or [])
        if len(waits) > 1:
            drain_inst.ins.sync_info = mybir.SyncInfo(
                on_wait=waits[:1], on_update=list(si.on_update or [])
            )
            for i in range(1, len(waits)):
                extra = nc.sync.drain()
                extra.ins.sync_info = mybir.SyncInfo(
                    on_wait=waits[i : i + 1], on_update=[]
                )
        nc.all_engine_barrier()
        assert self.sems is not None
        popped = nc._tile_sem_poison_stack.pop()
        assert popped is self._sem_poison
        nc.clear_and_free_semaphores(list(self.sems.allocated().values()))
        nc.all_engine_barrier()

    tile_mod.TileContext._drain_and_barrier = _drain_and_barrier
    tile_mod.TileContext._drain_split_patch = True


def _patch_birsim_off():
    """The walrus BIR-simulation pass re-executes every instruction on host
    and dominates compile time (~19 min for this kernel vs <1 s without).
    It is a validation-only pass; disable it for our compiles."""
    import concourse.bass_utils as bu

    if getattr(bu, "_birsim_off_patch", False):
        return
    orig = bu.bir_verify_and_optimise

    def patched(tmpdir, inp="bir.json", outp="file.neff", arch=None, *, dve_root=None):
        real_run = bu.run_command

        def run_hook(cmd, **kw):
            cmd = [
                "--enable-birsim=false" if c == "--enable-birsim=true" else c
                for c in cmd
            ]
            return real_run(cmd, **kw)

        bu.run_command = run_hook
        try:
            return orig(tmpdir, inp, outp, arch, dve_root=dve_root)
        finally:
            bu.run_command = real_run

    bu.bir_verify_and_optimise = patched
    bu._birsim_off_patch = True


def _build_nc():
    _patch_drain()
    _patch_birsim_off()
    nc = bass.Bass("TRN2", target_bir_lowering=False, debug=False)

    # token t of a slice = (partition t//4, chunk t%4)
    x_d = nc.declare_dram_parameter("x", [TPC, 128, 4, F], F32, isOutput=False)
    w_d = nc.declare_dram_parameter("w", [NK, 128, 128], F16, isOutput=False)
    id_d = nc.declare_dram_parameter("ident", [128, 128], F16, isOutput=False)
    o_d = nc.declare_dram_parameter("out", [2, TPC, 512, 64], F16, isOutput=True)

    with TileContext(nc) as tc:
        with (
            tc.tile_pool(name="const", bufs=1) as cpool,
            tc.tile_pool(name="nat", bufs=3) as natp,
            tc.tile_pool(name="xt", bufs=6) as xtp,
            tc.tile_pool(name="stg", bufs=2) as stgp,
            tc.tile_pool(name="pxt", bufs=6, space="PSUM") as pxtp,
            tc.tile_pool(name="pmm", bufs=2, space="PSUM") as pmmp,
        ):
            wsb = cpool.tile([128, NK, 128], F16, tag="w", name="w")
            nc.sync.dma_start(
                out=wsb[:, :, :], in_=w_d.rearrange("k p d -> p k d")
            )
            ident = cpool.tile([128, 128], F16, tag="ident", name="ident")
            nc.sync.dma_start(out=ident[:, :], in_=id_d[:, :])

            def load_quad(qd):
                nat = natp.tile([128, 4, 4, FW2], F16, tag="nat", name="nat")
                # SWDGE casting DMA: fp32 HBM -> fp16 SBUF during the transfer
                # (per slice: SWDGE can only balance <=3-dim access patterns)
                for t4 in range(4):
                    nc.gpsimd.dma_start(
                        out=nat[:, t4, :, 0:F], in_=x_d[4 * qd + t4]
                    )
                # col 362 = raw paired-slice feature 0 (haar is folded into W:
                # row0 = [.5Wg0|.5Wh0], row362 = [.5Wg0|-.5Wh0]); col 363 =
                # ones (bias); 364.. = zero K padding
                for pp in range(2):
                    e, o = 2 * pp, 2 * pp + 1
                    nc.vector.tensor_copy(nat[:, e, :, F], nat[:, o, :, 0])
                    nc.vector.tensor_copy(nat[:, o, :, F], nat[:, e, :, 0])
                nc.gpsimd.memset(nat[:, :, :, F + 1 : F + 2], 1.0)
                nc.gpsimd.memset(nat[:, :, :, F + 2 : FW2], 0.0)
                return nat

            def trans_phase(nath, ti):
                xts = []
                for k in range(NK):
                    pxt = pxtp.tile([128, 512], F16, tag="pxt", name="pxt")
                    for c in range(4):
                        nc.tensor.transpose(
                            pxt[:, c * 128 : (c + 1) * 128],
                            nath[:, ti, c, k * 128 : (k + 1) * 128],
                            ident[:, :],
                        )
                    xtb = xtp.tile([128, 512], F16, tag="xt", name="xt")
                    if k == 1:
                        nc.scalar.copy(xtb[:, :], pxt[:, :])
                    else:
                        nc.vector.tensor_copy(xtb[:, :], pxt[:, :])
                    xts.append(xtb)
                return xts

            def gemm_phase(xts, stg, ti):
                pmm = pmmp.tile([128, 4, 128], F32, tag="pmm", name="pmm")
                for c in range(4):
                    for k in range(NK):
                        nc.tensor.matmul(
                            pmm[:, c, :],
                            xts[k][:, c * 128 : (c + 1) * 128],
                            wsb[:, k, :],
                            start=(k == 0),
                            stop=(k == NK - 1),
                        )
                nc.scalar.copy(
                    stg[:, ti],
                    pmm.rearrange("p q (lh d) -> p lh q d", lh=2),
                )

            def store_quad(qd, stg):
                for lh in range(2):
                    nc.scalar.dma_start(
                        out=o_d[lh, 4 * qd : 4 * qd + 4].rearrange(
                            "t (p q) d -> p t q d", q=4
                        ),
                        in_=stg[:, :, lh],
                    )

            # software pipeline: PE transposes slice t while slice t-1's GEMM
            # waits on its PSUM->SBUF casts
            stg_tiles = {}
            prev = None
            for t in range(TPC):
                qd, ti = divmod(t, 4)
                if ti == 0:
                    nath = load_quad(qd)
                    stg_tiles[qd] = stgp.tile(
                        [128, 4, 2, 4, 64], F16, tag="stg", name="stg"
                    )
                xts = trans_phase(nath, ti)
                if prev is not None:
                    pq, pt = divmod(t - 1, 4)
                    gemm_phase(prev, stg_tiles[pq], pt)
                    if pt == 3:
                        store_quad(pq, stg_tiles.pop(pq))
                prev = xts
            gemm_phase(prev, stg_tiles[TPC // 4 - 1], 3)
            store_quad(TPC // 4 - 1, stg_tiles.pop(TPC // 4 - 1))
    return nc


_NC = None


def kernel(x, Wg_w, Wg_b, Wh_w, Wh_b):
    global _NC
    if _NC is None:
        _NC = _build_nc()

    x = np.ascontiguousarray(np.asarray(x, dtype=np.float32))
    Wg_w = np.asarray(Wg_w, dtype=np.float32)
    Wg_b = np.asarray(Wg_b, dtype=np.float32)
    Wh_w = np.asarray(Wh_w, dtype=np.float32)
    Wh_b = np.asarray(Wh_b, dtype=np.float32)

    waug = np.zeros((FW2, 128), dtype=np.float32)
    waug[:F, :64] = Wg_w.T
    waug[:F, 64:] = Wh_w.T
    waug[0, :64] = 0.5 * Wg_w[:, 0]
    waug[0, 64:] = 0.5 * Wh_w[:, 0]
    waug[F, :64] = 0.5 * Wg_w[:, 0]
    waug[F, 64:] = -0.5 * Wh_w[:, 0]
    waug[F + 1, :64] = Wg_b
    waug[F + 1, 64:] = Wh_b
    waug = waug.reshape(NK, 128, 128).astype(np.float16)
    ident = np.eye(128, dtype=np.float16)

    xf = x.reshape(BS, N, F)
    in_maps = []
    for i in range(N_CORES):
        shard = xf[i * TPC : (i + 1) * TPC].reshape(TPC, 128, 4, F)
        in_maps.append({"x": shard, "w": waug, "ident": ident})

    res = run_bass_kernel_spmd(_NC, in_maps, list(range(N_CORES)))
    out_l = np.concatenate(
        [res.results[i]["out"][0] for i in range(N_CORES)], axis=0
    ).astype(np.float32).reshape(B, S, N, D)
    out_h = np.concatenate(
        [res.results[i]["out"][1] for i in range(N_CORES)], axis=0
    ).astype(np.float32).reshape(B, S, N, D)
    return (out_l, out_h)


# revision 10
# speedup vs baseline: 2.3946x; 1.1708x over previous
"""Trainium2 Bass kernel for nn_DecouplingFlowLayer.

Computes, for x [B=4, S=128, N=512, F=362] fp32:
  X_l_proj = (x with feature0 := Haar-lowpass)  @ Wg^T + Wg_b   -> [B,S,N,64]
  X_h_proj = (x with feature0 := Haar-highpass) @ Wh^T + Wh_b   -> [B,S,N,64]

Strategy (per NeuronCore, data-parallel over B*S across 8 cores), v2:
  - One "tile" = one (b,s) slice = 512 tokens x 362 features (contiguous in
    HBM).  Token t of a slice sits at SBUF partition t//4, chunk t%4 so that
    output stores stay >=512B contiguous per partition.
  - The whole datapath is fp16 (tolerance is 2e-2; fp16 keeps us ~1e-3):
      * the x load is a gpsimd (SWDGE) casting DMA: fp32 HBM -> fp16 SBUF
        during the transfer, so no on-chip convert pass exists at all.
      * fp16 PE transposes run 1 cyc/row (fp32: 2) and their LDWEIGHTS get
        the compiler's fast-weight-load path (fp16 + 128 cols).
  - K is padded 362 -> 384: col 362 = the paired slice's RAW feature 0 (the
    Haar avg/diff algebra is folded into W rows 0/362, uniformly for even
    and odd slices), col 363 = ones (fuses the bias into the GEMM),
    cols 364.. = zeros.
  - GEMM is "flipped": stationary = transposed x chunk [f,128tok], moving =
    W block [f,128] -> PSUM [tok, d] directly; no output transpose and a
    single PSUM->SBUF fp16 copy per slice.
  - Outputs are stored fp16 (halves write traffic; DMA is the roofline);
    the host widens to fp32.
"""

import numpy as np

import concourse.bass as bass
import concourse.mybir as mybir
from concourse.bass_utils import run_bass_kernel_spmd
from concourse.tile import TileContext

F32 = mybir.dt.float32
F16 = mybir.dt.float16

N_CORES = 8
B, S, N, F, D = 4, 128, 512, 362, 64
BS = B * S                     # 512 (b,s) slices
TPC = BS // N_CORES            # 64 slices per core
FW2 = 384                      # padded K: 362 data + 2 haar + 1 ones + 19 zero
NK = 3                         # K blocks of 128


def _patch_drain():
    """walrus (TRN2) can encode only one sync-wait per instruction for several
    instruction formats (Matmult/S3_LW, SP CTRL drain, ...). Tile's scheduler
    happily attaches 2+ waits. Hoist excess waits onto standalone
    InstEventSemaphore instructions on the same engine (identical sequencer
    stall semantics), keeping one wait on the original instruction."""
    import concourse.tile as tile_mod
    from concourse.vector_clock import ScopedClock

    if getattr(tile_mod.TileContext, "_drain_split_patch", False):
        return

    orig_cal = tile_mod.TileContext._commit_and_lower

    def _commit_and_lower(self, inst, original_block, old_bb_map, bb_to_exit_bb):
        si = getattr(inst, "sync_info", None)
        waits = list(si.on_wait) if (si and si.on_wait) else []
        if (
            len(waits) > 1
            and isinstance(inst, mybir.Instruction)
            and inst.engine != mybir.EngineType.Unassigned
            and not type(inst).__name__.startswith("BassTile")
        ):
            for w in waits[:-1]:
                ev = mybir.InstEventSemaphore(
                    name=f"EVW-{self.nc.next_id()}",
                    ins=[],
                    outs=[],
                    sync_info=mybir.SyncInfo(on_wait=[w], on_update=[]),
                )
                ev.engine = inst.engine
                orig_cal(self, ev, original_block, old_bb_map, bb_to_exit_bb)
            inst.sync_info = mybir.SyncInfo(
                on_wait=[waits[-1]], on_update=list(si.on_update or [])
            )
        return orig_cal(self, inst, original_block, old_bb_map, bb_to_exit_bb)

    tile_mod.TileContext._commit_and_lower = _commit_and_lower

    def _drain_and_barrier(self, tick_clock, wait_clock):
        nc = self.nc
        drain_inst = nc.sync.drain()
        wait_clock.add_sem_waits(
            drain_inst.ins, ScopedClock({None: tick_clock.global_clock})
        )
        si = drain_inst.ins.sync_info
        waits = list(si.on_wait or [])
        if len(waits) > 1:
            drain_inst.ins.sync_info = mybir.SyncInfo(
                on_wait=waits[:1], on_update=list(si.on_update or [])
            )
            for i in range(1, len(waits)):
                extra = nc.sync.drain()
                extra.ins.sync_info = mybir.SyncInfo(
                    on_wait=waits[i : i + 1], on_update=[]
                )
        nc.all_engine_barrier()
        assert self.sems is not None
        popped = nc._tile_sem_poison_stack.pop()
        assert popped is self._sem_poison
        nc.clear_and_free_semaphores(list(self.sems.allocated().values()))
        nc.all_engine_barrier()

    tile_mod.TileContext._drain_and_barrier = _drain_and_barrier
    tile_mod.TileContext._drain_split_patch = True


def _patch_birsim_off():
    """The walrus BIR-simulation pass re-executes every instruction on host
    and dominates compile time (~19 min for this kernel vs <1 s without).
    It is a validation-only pass; disable it for our compiles."""
    import concourse.bass_utils as bu

    if getattr(bu, "_birsim_off_patch", False):
        return
    orig = bu.bir_verify_and_optimise

    def patched(tmpdir, inp="bir.json", outp="file.neff", arch=None, *, dve_root=None):
        real_run = bu.run_command

        def run_hook(cmd, **kw):
            cmd = [
                "--enable-birsim=false" if c == "--enable-birsim=true" else c
                for c in cmd
            ]
            return real_run(cmd, **kw)

        bu.run_command = run_hook
        try:
            return orig(tmpdir, inp, outp, arch, dve_root=dve_root)
        finally:
            bu.run_command = real_run

    bu.bir_verify_and_optimise = patched
    bu._birsim_off_patch = True


def _build_nc():
    _patch_drain()
    _patch_birsim_off()
    nc = bass.Bass("TRN2", target_bir_lowering=False, debug=False)

    # token t of a slice = (partition t//4, chunk t%4)
    x_d = nc.declare_dram_parameter("x", [TPC, 128, 4, F], F32, isOutput=False)
    w_d = nc.declare_dram_parameter("w", [NK, 128, 128], F16, isOutput=False)
    id_d = nc.declare_dram_parameter("ident", [128, 128], F32, isOutput=False)
    o_d = nc.declare_dram_parameter("out", [2, TPC, 512, 64], F16, isOutput=True)

    with TileContext(nc) as tc:
        with (
            tc.tile_pool(name="const", bufs=1) as cpool,
            tc.tile_pool(name="nat", bufs=3) as natp,
            tc.tile_pool(name="xt", bufs=6) as xtp,
            tc.tile_pool(name="stg", bufs=2) as stgp,
            tc.tile_pool(name="pxt", bufs=6, space="PSUM") as pxtp,
            tc.tile_pool(name="pmm", bufs=2, space="PSUM") as pmmp,
        ):
            wsb = cpool.tile([128, NK, 128], F16, tag="w", name="w")
            nc.sync.dma_start(
                out=wsb[:, :, :], in_=w_d.rearrange("k p d -> p k d")
            )
            ident = cpool.tile([128, 128], F32, tag="ident", name="ident")
            nc.sync.dma_start(out=ident[:, :], in_=id_d[:, :])

            def load_quad(qd):
                # fp32 HWDGE load (full ring rate; the SWDGE cast path costs
                # ~20% ring throughput) — fp16 conversion happens for free in
                # the PSUM->SBUF copies after the PE transposes
                nat = natp.tile([128, 4, 4, FW2], F32, tag="nat", name="nat")
                # per slice: the token remap breaks (t,c) stride merging, and
                # DMA AP balancing is limited to 3 dims
                for t4 in range(4):
                    nc.sync.dma_start(
                        out=nat[:, t4, :, 0:F], in_=x_d[4 * qd + t4]
                    )
                # col 362 = raw paired-slice feature 0 (haar is folded into W:
                # row0 = [.5Wg0|.5Wh0], row362 = [.5Wg0|-.5Wh0]); col 363 =
                # ones (bias); 364.. = zero K padding
                for pp in range(2):
                    e, o = 2 * pp, 2 * pp + 1
                    nc.vector.tensor_copy(nat[:, e, :, F], nat[:, o, :, 0])
                    nc.vector.tensor_copy(nat[:, o, :, F], nat[:, e, :, 0])
                nc.gpsimd.memset(nat[:, :, :, F + 1 : F + 2], 1.0)
                nc.gpsimd.memset(nat[:, :, :, F + 2 : FW2], 0.0)
                return nat

            def trans_phase(nath, ti):
                xts = []
                for k in range(NK):
                    pxt = pxtp.tile([128, 512], F32, tag="pxt", name="pxt")
                    for c in range(4):
                        nc.tensor.transpose(
                            pxt[:, c * 128 : (c + 1) * 128],
                            nath[:, ti, c, k * 128 : (k + 1) * 128],
                            ident[:, :],
                        )
                    xtb = xtp.tile([128, 512], F16, tag="xt", name="xt")
                    if k == 1:
                        nc.scalar.copy(xtb[:, :], pxt[:, :])
                    else:
                        nc.vector.tensor_copy(xtb[:, :], pxt[:, :])
                    xts.append(xtb)
                return xts

            def gemm_phase(xts, stg, ti):
                pmm = pmmp.tile([128, 4, 128], F32, tag="pmm", name="pmm")
                for c in range(4):
                    for k in range(NK):
                        nc.tensor.matmul(
                            pmm[:, c, :],
                            xts[k][:, c * 128 : (c + 1) * 128],
                            wsb[:, k, :],
                            start=(k == 0),
                            stop=(k == NK - 1),
                        )
                nc.scalar.copy(
                    stg[:, ti],
                    pmm.rearrange("p q (lh d) -> p lh q d", lh=2),
                )

            def store_quad(qd, stg):
                for lh in range(2):
                    nc.scalar.dma_start(
                        out=o_d[lh, 4 * qd : 4 * qd + 4].rearrange(
                            "t (p q) d -> p t q d", q=4
                        ),
                        in_=stg[:, :, lh],
                    )

            # software pipeline: PE transposes slice t while slice t-1's GEMM
            # waits on its PSUM->SBUF casts
            stg_tiles = {}
            prev = None
            for t in range(TPC):
                qd, ti = divmod(t, 4)
                if ti == 0:
                    nath = load_quad(qd)
                    stg_tiles[qd] = stgp.tile(
                        [128, 4, 2, 4, 64], F16, tag="stg", name="stg"
                    )
                xts = trans_phase(nath, ti)
                if prev is not None:
                    pq, pt = divmod(t - 1, 4)
                    gemm_phase(prev, stg_tiles[pq], pt)
                    if pt == 3:
                        store_quad(pq, stg_tiles.pop(pq))
                prev = xts
            gemm_phase(prev, stg_tiles[TPC // 4 - 1], 3)
            store_quad(TPC // 4 - 1, stg_tiles.pop(TPC // 4 - 1))
    return nc


_NC = None


def kernel(x, Wg_w, Wg_b, Wh_w, Wh_b):
    global _NC
    if _NC is None:
        _NC = _build_nc()

    x = np.ascontiguousarray(np.asarray(x, dtype=np.float32))
    Wg_w = np.asarray(Wg_w, dtype=np.float32)
    Wg_b = np.asarray(Wg_b, dtype=np.float32)
    Wh_w = np.asarray(Wh_w, dtype=np.float32)
    Wh_b = np.asarray(Wh_b, dtype=np.float32)

    waug = np.zeros((FW2, 128), dtype=np.float32)
    waug[:F, :64] = Wg_w.T
    waug[:F, 64:] = Wh_w.T
    waug[0, :64] = 0.5 * Wg_w[:, 0]
    waug[0, 64:] = 0.5 * Wh_w[:, 0]
    waug[F, :64] = 0.5 * Wg_w[:, 0]
    waug[F, 64:] = -0.5 * Wh_w[:, 0]
    waug[F + 1, :64] = Wg_b
    waug[F + 1, 64:] = Wh_b
    waug = waug.reshape(NK, 128, 128).astype(np.float16)
    ident = np.eye(128, dtype=np.float32)

    xf = x.reshape(BS, N, F)
    in_maps = []
    for i in range(N_CORES):
        shard = xf[i * TPC : (i + 1) * TPC].reshape(TPC, 128, 4, F)
        in_maps.append({"x": shard, "w": waug, "ident": ident})

    res = run_bass_kernel_spmd(_NC, in_maps, list(range(N_CORES)))
    out_l = np.concatenate(
        [res.results[i]["out"][0] for i in range(N_CORES)], axis=0
    ).astype(np.float32).reshape(B, S, N, D)
    out_h = np.concatenate(
        [res.results[i]["out"][1] for i in range(N_CORES)], axis=0
    ).astype(np.float32).reshape(B, S, N, D)
    return (out_l, out_h)
